# revision 13
# baseline (speedup 1.0000x reference)
"""Trainium2 Bass kernel for nn_Corr_Layer (B,C,F,T = 256,8,8,4096).

reference:
    common[b,t] = sum_{c,f'} W[c,f'+1] * x[b,c,f',t]
    per[b,f,t]  = sum_c     W[c,0]    * x[b,c,f,t]
    corr        = per + common + b0
    out         = concat([x, corr[:,None]], axis=1)   # [B, 9, F, T]

Strategy (pure data parallel over batch, 32 batches per core):
  - For each batch, output rows (ch*F+f) 0..63 are a verbatim copy of x[b]
    and rows 64..71 are corr[b].  So per batch the output is one contiguous
    [72, T] block: [x[b] (64 rows); corr[b] (8 rows)].
  - corr[b] = M @ x[b]  with M[f, c*8+f'] = W[c,0]*delta(f,f') + W[c,f'+1],
    computed on the TensorEngine.  Two batches are packed per SBUF tile
    [128, T] and GROUPS such pairs accumulate into one [16*GROUPS, 512]
    PSUM chunk via zero-padded block lhsT matrices, so corr for 2*GROUPS
    batches lands on many partitions -> wide, DMA-efficient stores.
  - HBM traffic per core: read 32 MiB + write 36 MiB (roofline ~200 us).
"""

import numpy as np

B, C, F, T = 256, 8, 8, 4096
NCORES = 8
BPC = B // NCORES        # 32 batches per core
ROWS = C * F             # 64 x-rows per batch
OROWS = ROWS + F         # 72 output rows per batch
NFREE = 512              # PSUM bank free size (fp32)
NCHUNK = T // NFREE      # 8

# build-time tunables (defaults = best TimelineSim config: 201.9 us/core,
# 98% of the ~198.5 us DMA roofline for 68 MiB of HBM traffic per core)
CFG = {
    "groups": 4,        # batch-pairs accumulated per PSUM chunk
    "order": "jmajor",  # 'jmajor' (chunk-major) or 'gmajor' (pair-major)
    "corr_splits": 4,   # number of DMAs for each round's corr store
    "mm_dtype": "float32",  # 'float32' or 'float32r'
    "xp_bufs": None,    # default 2*groups
    "ps_bufs": None,    # default min(8, 2*NCHUNK...)
    "store_eng": "scalar",  # stores on ACT HWDGE: separate queues from loads
    "w_eng": "gpsimd",  # small weight/bias loads on SWDGE, off the load queues
}

_NC_CACHE = {}


def _build_nc():
    import concourse.bacc as bacc
    import concourse.mybir as mybir
    from concourse.tile import TileContext

    groups = CFG["groups"]
    rounds = BPC // (2 * groups)
    corr_p = 16 * groups                # corr partitions per round
    f32 = mybir.dt.float32
    mm_dt = getattr(mybir.dt, CFG["mm_dtype"])
    xp_bufs = CFG["xp_bufs"] or 2 * groups
    ps_bufs = CFG["ps_bufs"] or (NCHUNK if CFG["order"] == "gmajor" else 4)

    nc = bacc.Bacc(None, target_bir_lowering=False, debug=False)

    x_in = nc.declare_dram_parameter("x", [BPC * ROWS, T], f32, isOutput=False)
    w_in = nc.declare_dram_parameter("lhsT", [128, groups * corr_p], f32, isOutput=False)
    b_in = nc.declare_dram_parameter("bvec", [128, 1], f32, isOutput=False)
    out = nc.declare_dram_parameter("out", [BPC, OROWS, T], f32, isOutput=True)

    with TileContext(nc) as tc:
        with (
            tc.tile_pool(name="xp", bufs=xp_bufs) as xp,
            tc.tile_pool(name="cp", bufs=2) as cp,
            tc.tile_pool(name="wp", bufs=1) as wp,
            tc.tile_pool(name="ps", bufs=ps_bufs, space="PSUM") as ps,
        ):
            weng = getattr(nc, CFG["w_eng"])
            wt = wp.tile([128, groups * corr_p], f32)
            weng.dma_start(out=wt[:], in_=w_in[:])
            bt = wp.tile([128, 1], f32)
            weng.dma_start(out=bt[:], in_=b_in[:])

            for r in range(rounds):
                xtiles = []
                for g in range(groups):
                    xt = xp.tile([128, T], f32, name=f"xt_{r}_{g}", tag="xt")
                    row0 = (r * groups + g) * 128
                    nc.sync.dma_start(out=xt[:], in_=x_in[row0 : row0 + 128, :])
                    xtiles.append(xt)

                psums = [
                    ps.tile([corr_p, NFREE], f32, name=f"pt_{r}_{j}", tag="pt")
                    for j in range(NCHUNK)
                ]

                def mm(j, g):
                    lhs = wt[:, corr_p * g : corr_p * (g + 1)]
                    rhs = xtiles[g][:, NFREE * j : NFREE * (j + 1)]
                    if mm_dt != f32:
                        lhs = lhs.bitcast(mm_dt)
                        rhs = rhs.bitcast(mm_dt)
                    nc.tensor.matmul(
                        psums[j][:],
                        lhs,
                        rhs,
                        start=(g == 0),
                        stop=(g == groups - 1),
                    )

                corr = cp.tile([corr_p, T], f32, name=f"corr_{r}", tag="corr")

                def act(j):
                    nc.scalar.activation(
                        corr[:, NFREE * j : NFREE * (j + 1)],
                        psums[j][:],
                        mybir.ActivationFunctionType.Identity,
                        bias=bt[0:corr_p],
                    )

                if CFG["order"] == "jmajor":
                    for j in range(NCHUNK):
                        for g in range(groups):
                            mm(j, g)
                        act(j)
                else:
                    for g in range(groups):
                        for j in range(NCHUNK):
                            mm(j, g)
                    for j in range(NCHUNK):
                        act(j)

                st = getattr(nc, CFG["store_eng"])
                for g in range(groups):
                    b0 = (r * groups + g) * 2
                    # [128, T] sbuf -> [2, 64, T] dram: same element order
                    st.dma_start(
                        out=out[b0 : b0 + 2, 0:ROWS, :], in_=xtiles[g][:]
                    )
                # corr [corr_p, T] sbuf -> [2*groups, 8, T] dram slab, in
                # corr_splits column chunks (earlier chunks store while later
                # chunks still compute)
                nsp = CFG["corr_splits"]
                cw = T // nsp
                bb = r * 2 * groups
                for s in range(nsp):
                    st.dma_start(
                        out=out[bb : bb + 2 * groups, ROWS:OROWS, s * cw : (s + 1) * cw],
                        in_=corr[:, s * cw : (s + 1) * cw],
                    )

    nc.compile()
    return nc


def _get_nc():
    key = tuple(sorted(CFG.items()))
    if key not in _NC_CACHE:
        _NC_CACHE[key] = _build_nc()
    return _NC_CACHE[key]


def _prep_small(W, b):
    W = np.asarray(W, dtype=np.float32)
    b = np.asarray(b, dtype=np.float32).reshape(-1)
    groups = CFG["groups"]
    corr_p = 16 * groups
    # A[c*8+f', f] = W[c, f'+1] + delta(f,f') * W[c, 0]
    A = np.zeros((ROWS, F), dtype=np.float32)
    for c in range(C):
        for fp in range(F):
            A[c * F + fp, :] = W[c, fp + 1]
            A[c * F + fp, fp] += W[c, 0]
    # block-diagonal over a pair of batches: [128, 16]
    A_pair = np.zeros((128, 16), dtype=np.float32)
    A_pair[0:ROWS, 0:F] = A
    A_pair[ROWS:128, F:16] = A
    # one zero-padded [128, corr_p] block per group g, packed side by side
    lhsT = np.zeros((128, groups * corr_p), dtype=np.float32)
    for g in range(groups):
        lhsT[:, corr_p * g + 16 * g : corr_p * g + 16 * g + 16] = A_pair
    bvec = np.full((128, 1), b[0], dtype=np.float32)
    return lhsT, bvec


def _run(x, W, b, **spmd_kwargs):
    from concourse.bass_utils import run_bass_kernel_spmd

    x = np.ascontiguousarray(np.asarray(x, dtype=np.float32))
    assert x.shape == (B, C, F, T), x.shape
    lhsT, bvec = _prep_small(W, b)

    xf = x.reshape(B * ROWS, T)
    rows_pc = BPC * ROWS
    in_maps = [
        {"x": xf[i * rows_pc : (i + 1) * rows_pc], "lhsT": lhsT, "bvec": bvec}
        for i in range(NCORES)
    ]
    nc = _get_nc()
    res = run_bass_kernel_spmd(nc, in_maps, list(range(NCORES)), **spmd_kwargs)
    shards = [res.results[i]["out"] for i in range(NCORES)]
    full = np.concatenate(shards, axis=0)  # [B, 72, T]
    return full.reshape(B, C + 1, F, T), res


def kernel(x, W, b):
    out, _ = _run(x, W, b)
    return out
